# revision 2
# baseline (speedup 1.0000x reference)
"""Trainium2 Bass kernel for the DiCNN (WaveNet-like) module.

Strategy: pure data parallelism — 4 batch items per core on 8 cores.
On-chip layout: channels on partitions, time on the free dim. The four
batch items of a core are stacked as 4x32-partition bands for the
HID=32 layers (block-diagonal weights), and as 2x64 bands for the
64-channel causal layer (two batch-pair passes accumulated into one
PSUM tile at different output bands).

The final 32->448 1x1 conv is computed "flipped": stationary = s1 data
chunk [33,128] (incl. a constant-ones row for the bias fold), moving =
w_sk2^T (+bias row) [33,448], so PSUM holds [t,co] — the exact output
layout — and is DMA'd out with no transpose.

All matmul operands are bf16 (PSUM accumulation fp32). x is cast to
bf16 host-side so the input load can use the HWDGE DMA-transpose path.
"""

import numpy as np
import ml_dtypes

import concourse.bacc as bacc
import concourse.tile as tile
from concourse import mybir
from concourse.bass_utils import run_bass_kernel_spmd

BF16 = mybir.dt.bfloat16
FP32 = mybir.dt.float32

B, T, C_IN, HID, C_OUT, K = 32, 4096, 64, 32, 448, 2
N_CORES = 8
BPC = B // N_CORES          # batches per core = 4
TT = 512                    # time-tile size
NT = T // TT                # 8 tiles
XROWS = 4112                # 4097 rounded up to a multiple of 16 (xbar tile rows)

AF = mybir.ActivationFunctionType
ALU = mybir.AluOpType

_cached_nc = None


def _f(x):
    return np.asarray(x, dtype=np.float32)


def _bf(x):
    return np.asarray(x, dtype=np.float32).astype(ml_dtypes.bfloat16)


def _tile4(v):
    # [32] -> [128,1] per-partition vector tiled over the 4 batch bands
    return np.tile(_f(v).reshape(-1), 4).reshape(128, 1)


def prepare_weights(w_causal, b_causal, wd0, bd0, ws0, bs0, wo0, bo0,
                    wd1, bd1, ws1, bs1, wo1, bo1, w_sk1, b_sk1, w_sk2, b_sk2):
    """Host-side weight layout transforms (shared across all cores)."""
    del wo1, bo1  # dead code: z after the last block is never used

    def diag4(w32):  # w32: [32,32] (in,out) -> block-diag [128,128]
        s = np.zeros((128, 128), np.float32)
        for i in range(4):
            s[32 * i:32 * i + 32, 32 * i:32 * i + 32] = w32
        return s

    # causal conv stationaries: [pair p, tap k] -> [128,128]
    wc = np.zeros((128, 4, 128), np.float32)
    for p in range(2):
        for k in range(2):
            wcT = _f(w_causal)[:, :, k].T  # [64,32] (in,out)
            s = np.zeros((128, 128), np.float32)
            s[0:64, 64 * p:64 * p + 32] = wcT          # batch 2p   -> band 2p
            s[64:128, 64 * p + 32:64 * p + 64] = wcT   # batch 2p+1 -> band 2p+1
            wc[:, 2 * p + k, :] = s

    # dilated conv stationaries: [block b, tap k] -> diag4
    wd = np.zeros((128, 4, 128), np.float32)
    for blk, w in enumerate((wd0, wd1)):
        for k in range(2):
            wd[:, 2 * blk + k, :] = diag4(_f(w)[:, :, k].T)

    wsr = np.zeros((128, 2, 128), np.float32)
    wsr[:, 0, :] = diag4(_f(ws0)[:, :, 0].T)
    wsr[:, 1, :] = diag4(_f(wo0)[:, :, 0].T)
    ws1d = diag4(_f(ws1)[:, :, 0].T)

    # w_sk1 stationaries, one per batch band: [128, 4, 33]
    wsk1 = np.zeros((128, 4, 33), np.float32)
    w1T = _f(w_sk1)[:, :, 0].T  # [32,32]
    for i in range(4):
        wsk1[32 * i:32 * i + 32, i, 0:32] = w1T
        # col 32 stays 0 -> psum row 32 = 0; relu(0 + 1.0 bias) = 1.0 ones row

    # final moving operand: rows 0..31 = w_sk2^T, row 32 = bias
    w2 = np.zeros((33, 448), np.float32)
    w2[0:32, :] = _f(w_sk2)[:, :, 0].T
    w2[32, :] = _f(b_sk2)

    bvecs = np.zeros((128, 6), np.float32)
    bvecs[:, 0] = _tile4(b_causal)[:, 0]
    bvecs[:, 1] = _tile4(bd0)[:, 0]
    bvecs[:, 2] = _tile4(bd1)[:, 0]
    bvecs[:, 3] = _tile4(bo0)[:, 0]
    bvecs[:, 4] = _tile4(_f(bs0) + _f(bs1))[:, 0]
    bvecs[0:32, 5] = _f(b_sk1)
    bvecs[32, 5] = 1.0

    return dict(
        wc=_bf(wc), wd=_bf(wd), wsr=_bf(wsr), ws1d=_bf(ws1d),
        wsk1=_bf(wsk1), w2=_bf(w2), bvecs=np.ascontiguousarray(bvecs),
    )


def prepare_x(x, core):
    """Per-core transposed-input staging array [2, XROWS, 128] bf16.

    Row 0 is the causal zero pad (t=-1); row 1+t holds x[b, t, :] for the
    two batches of pair p side by side on the channel axis.
    """
    xT = np.zeros((2, XROWS, 128), ml_dtypes.bfloat16)
    xb = _bf(x)
    for p in range(2):
        xT[p, 1:1 + T, 0:64] = xb[4 * core + 2 * p]
        xT[p, 1:1 + T, 64:128] = xb[4 * core + 2 * p + 1]
    return xT


def build_nc():
    nc = bacc.Bacc("TRN2", target_bir_lowering=False, debug=False,
                   num_devices=N_CORES)

    xT_d = nc.dram_tensor("xT", [2, XROWS, 128], BF16, kind="ExternalInput")
    wc_d = nc.dram_tensor("wc", [128, 4, 128], BF16, kind="ExternalInput")
    wd_d = nc.dram_tensor("wd", [128, 4, 128], BF16, kind="ExternalInput")
    wsr_d = nc.dram_tensor("wsr", [128, 2, 128], BF16, kind="ExternalInput")
    ws1_d = nc.dram_tensor("ws1d", [128, 128], BF16, kind="ExternalInput")
    wsk1_d = nc.dram_tensor("wsk1", [128, 4, 33], BF16, kind="ExternalInput")
    w2_d = nc.dram_tensor("w2", [33, 448], BF16, kind="ExternalInput")
    bv_d = nc.dram_tensor("bvecs", [128, 6], FP32, kind="ExternalInput")
    y_d = nc.dram_tensor("y", [BPC, T, C_OUT], FP32, kind="ExternalOutput")

    with tile.TileContext(nc) as tc:
        with (
            tc.tile_pool(name="const", bufs=1) as const,
            tc.tile_pool(name="persist", bufs=1) as persist,
            tc.tile_pool(name="act", bufs=2) as actp,
            tc.tile_pool(name="gbuf", bufs=2) as gbuf,
            tc.tile_pool(name="s1buf", bufs=2) as s1buf,
            tc.tile_pool(name="outbuf", bufs=6) as outbuf,
            tc.tile_pool(name="pz", bufs=2, space="PSUM") as pzp,
            tc.tile_pool(name="pg", bufs=2, space="PSUM") as pgp,
            tc.tile_pool(name="pskip", bufs=1, space="PSUM") as pskipp,
            tc.tile_pool(name="pout", bufs=3, space="PSUM") as poutp,
        ):
            # ---- constants to SBUF ----
            wc_s = const.tile([128, 4, 128], BF16)
            nc.sync.dma_start(wc_s[:], wc_d.ap())
            wd_s = const.tile([128, 4, 128], BF16)
            nc.sync.dma_start(wd_s[:], wd_d.ap())
            wsr_s = const.tile([128, 2, 128], BF16)
            nc.sync.dma_start(wsr_s[:], wsr_d.ap())
            ws1_s = const.tile([128, 128], BF16)
            nc.sync.dma_start(ws1_s[:], ws1_d.ap())
            wsk1_s = const.tile([128, 4, 33], BF16)
            nc.sync.dma_start(wsk1_s[:], wsk1_d.ap())
            w2_s = const.tile([33, 448], BF16)
            nc.sync.dma_start(w2_s[:], w2_d.ap())
            bv_s = const.tile([128, 6], FP32)
            nc.sync.dma_start(bv_s[:], bv_d.ap())

            bcausal = bv_s[:, 0:1]
            bd_v = (bv_s[:, 1:2], bv_s[:, 2:3])
            bo0_v = bv_s[:, 3:4]
            bskip_v = bv_s[:, 4:5]
            bsk1_v = bv_s[0:33, 5:6]

            # ---- persistent activations ----
            x_s = [persist.tile([128, XROWS], BF16, tag=f"x{p}", name=f"x_s{p}")
                   for p in range(2)]
            for p in range(2):
                nc.sync.dma_start(x_s[p][:], xT_d[p], transpose=True)
            z0_s = persist.tile([128, 4100], BF16, tag="z0")
            nc.vector.memset(z0_s[:, 0:1], 0.0)
            z1_s = persist.tile([128, 4100], BF16, tag="z1")
            nc.vector.memset(z1_s[:, 0:2], 0.0)

            for it in range(NT):
                t0 = TT * it

                # ---- causal conv: 4 accumulating MMs -> one [128,512] psum
                pz = pzp.tile([128, TT], FP32)
                first = True
                for p in range(2):
                    rhs = (x_s[p][:, t0:t0 + TT], x_s[p][:, t0 + 1:t0 + 1 + TT])
                    for k in range(2):
                        nc.tensor.matmul(pz[:], wc_s[:, 2 * p + k, :], rhs[k],
                                         start=first, stop=(p == 1 and k == 1))
                        first = False
                # z0 = psum + b_causal (bf16)
                nc.vector.tensor_scalar_add(z0_s[:, 1 + t0:1 + t0 + TT], pz[:],
                                            bcausal)

                # ---- two gated residual blocks ----
                for blk in range(2):
                    zsrc = z0_s if blk == 0 else z1_s
                    off = 1 if blk == 0 else 2   # left zero-pad width
                    dil = 1 if blk == 0 else 2
                    pg = pgp.tile([128, TT], FP32, tag="pg")
                    nc.tensor.matmul(pg[:], wd_s[:, 2 * blk, :],
                                     zsrc[:, off + t0 - dil:off + t0 - dil + TT],
                                     start=True, stop=False)
                    nc.tensor.matmul(pg[:], wd_s[:, 2 * blk + 1, :],
                                     zsrc[:, off + t0:off + t0 + TT],
                                     start=False, stop=True)
                    a_t = actp.tile([128, TT], BF16, tag="a")
                    nc.scalar.activation(a_t[:], pg[:], AF.Tanh, bias=bd_v[blk])
                    b_t = actp.tile([128, TT], BF16, tag="b")
                    nc.scalar.activation(b_t[:], pg[:], AF.Sigmoid, bias=bd_v[blk])
                    g_t = gbuf.tile([128, TT], BF16, tag="g")
                    nc.vector.tensor_mul(g_t[:], a_t[:], b_t[:])

                    if blk == 0:
                        pskip = pskipp.tile([128, TT], FP32)
                        nc.tensor.matmul(pskip[:], wsr_s[:, 0, :], g_t[:],
                                         start=True, stop=False,
                                         skip_group_check=True)
                        pres = pgp.tile([128, TT], FP32, tag="pg")
                        nc.tensor.matmul(pres[:], wsr_s[:, 1, :], g_t[:],
                                         start=True, stop=True)
                        # z1 = (res + bo0) + z0   (one fused DVE op)
                        nc.vector.scalar_tensor_tensor(
                            z1_s[:, 2 + t0:2 + t0 + TT], pres[:], bo0_v,
                            z0_s[:, 1 + t0:1 + t0 + TT], ALU.add, ALU.add)
                    else:
                        nc.tensor.matmul(pskip[:], ws1_s[:], g_t[:],
                                         start=False, stop=True,
                                         skip_group_check=True)

                # ---- head: s0 = relu(skip + bias), s1 = relu(wsk1@s0 + bias)
                s0_t = gbuf.tile([128, TT], BF16, tag="s0")
                nc.vector.tensor_scalar(s0_t[:], pskip[:], bskip_v, 0.0,
                                        ALU.add, ALU.max)
                s1_ts = []
                for i in range(4):
                    ps1 = pgp.tile([128, TT], FP32, tag="pg")
                    nc.tensor.matmul(ps1[0:33, :], wsk1_s[:, i, :], s0_t[:],
                                     start=True, stop=True)
                    s1_t = s1buf.tile([33, TT], BF16, tag=f"s1_{i}")
                    nc.vector.tensor_scalar(s1_t[:], ps1[0:33, :], bsk1_v, 0.0,
                                            ALU.add, ALU.max)
                    s1_ts.append(s1_t)

                # ---- final flipped matmuls: [t,co] out, DMA straight out
                cnt = 0
                for b in range(4):
                    for j in range(4):
                        po = poutp.tile([128, C_OUT], FP32)
                        nc.tensor.matmul(po[:], s1_ts[b][:, 128 * j:128 * j + 128],
                                         w2_s[:], start=True, stop=True)
                        o_t = outbuf.tile([128, C_OUT], FP32)
                        if cnt % 4 == 3:
                            nc.vector.tensor_copy(o_t[:], po[:])
                        else:
                            nc.scalar.copy(o_t[:], po[:])
                        cnt += 1
                        nc.sync.dma_start(
                            y_d[b, t0 + 128 * j:t0 + 128 * j + 128, :], o_t[:])
    nc.compile()
    return nc


def get_nc():
    global _cached_nc
    if _cached_nc is None:
        _cached_nc = build_nc()
    return _cached_nc


def kernel(**inputs):
    nc = get_nc()
    w = prepare_weights(
        inputs["w_causal"], inputs["b_causal"],
        inputs["wd0"], inputs["bd0"], inputs["ws0"], inputs["bs0"],
        inputs["wo0"], inputs["bo0"],
        inputs["wd1"], inputs["bd1"], inputs["ws1"], inputs["bs1"],
        inputs["wo1"], inputs["bo1"],
        inputs["w_sk1"], inputs["b_sk1"], inputs["w_sk2"], inputs["b_sk2"])
    x = np.asarray(inputs["x"])
    in_maps = [{"xT": prepare_x(x, c), **w} for c in range(N_CORES)]
    res = run_bass_kernel_spmd(nc, in_maps, list(range(N_CORES)))
    out = np.concatenate([res.results[c]["y"] for c in range(N_CORES)], axis=0)
    return out.astype(np.float32)
